# revision 3
# baseline (speedup 1.0000x reference)
"""BiAttention kernel for TRN2, 8-core data parallel over batch.

Math per batch (P=[Lp,D] premise, H=[Lh,D] hypothesis, S = P @ H^T):
  A_r = softmax over h of S (masked h >= hlen)       -> ctx_h = A_r @ H   [Lp,D]
  A_c = softmax over p of S (masked p >= plen)       -> ctx_p = A_c^T @ P [Lh,D]
  r1 = [P - ctx_h, ctx_h * P]   -> output transposed to [Lp, B, 2D]
  r2 = [H - ctx_p, ctx_p * H]   -> [B, Lh, 2D]

Strategy per core (4 batches):
  - load P,H; PE-transpose to PT,HT (d on partitions)
  - S tiles [p,h] and T=S^T tiles [h,p] via matmul (float32r)
  - masked softmax prep in each layout: mask-add (DVE, fused PSUM evac),
    free-dim max (negated), exp with per-partition bias (ACT, accum_out sums)
  - PE-transpose exp matrices into matmul-ready layouts (softmax dim on
    partitions), ctx matmuls (float32r), 1/sum folded into fused
    scalar_tensor_tensor ops that emit [P-ctx, ctx*P] straight from PSUM.
"""

import numpy as np

import concourse.bass as bass
import concourse.tile as tile
from concourse import bacc, mybir, bass_utils
from concourse.alu_op_type import AluOpType
from concourse.bass import ts
from concourse.masks import make_identity

B, LP, LH, D = 32, 1024, 1024, 1024
N_CORES = 8
BL = B // N_CORES          # batches per core
NT = LP // 128             # 128-row tiles per matrix
FP32 = mybir.dt.float32
FP32R = mybir.dt.float32r
I32 = mybir.dt.int32
NEG_BIG = -1.0e30
MMDT = FP32R  # matmul operand dtype

AX = mybir.AxisListType.X
ALU = AluOpType
ACTF = mybir.ActivationFunctionType


def _mm(nc, out, lhsT, rhs, start, stop, fp32r=True):
    nc.tensor.matmul(out, lhsT, rhs, start=start, stop=stop)


def build_nc(scores_fp32r=True, ctx_fp32r=True, n_iters=1):
    nc = bacc.Bacc("TRN2", target_bir_lowering=False, debug=False)

    prem = nc.dram_tensor("prem", [BL, LP, D], FP32, kind="ExternalInput").ap()
    hyp = nc.dram_tensor("hyp", [BL, LH, D], FP32, kind="ExternalInput").ap()
    plens = nc.dram_tensor("plens", [1, BL], I32, kind="ExternalInput").ap()
    hlens = nc.dram_tensor("hlens", [1, BL], I32, kind="ExternalInput").ap()
    r1 = nc.dram_tensor("r1", [LP, BL, 2 * D], FP32, kind="ExternalOutput").ap()
    r2 = nc.dram_tensor("r2", [BL, LH, 2 * D], FP32, kind="ExternalOutput").ap()

    with tile.TileContext(nc) as tc:
        with (
            tc.tile_pool(name="const", bufs=1) as constp,
            tc.tile_pool(name="pa", bufs=20) as pool_a,      # nat P/H + E1/E2
            tc.tile_pool(name="pb", bufs=18) as pool_b,      # PT/HT + reloads
            tc.tile_pool(name="pre", bufs=4) as pool_pre,    # masked S/T + exp pre
            tc.tile_pool(name="maskp", bufs=3) as pool_mask,
            tc.tile_pool(name="small", bufs=24) as pool_sm,
            tc.tile_pool(name="stage", bufs=2) as pool_st,
            tc.tile_pool(name="ps", bufs=2, space="PSUM") as pool_ps,
            tc.tile_pool(name="pt", bufs=4, space="PSUM") as pool_pt,
        ):
            # one-time constants
            ident = constp.tile([128, 128], FP32)
            make_identity(nc, ident)
            iota_i = constp.tile([128, 1024], I32)
            nc.gpsimd.iota(iota_i, pattern=[[1, 1024]], base=0, channel_multiplier=0)
            iota_f = constp.tile([128, 1024], FP32)
            nc.vector.tensor_copy(iota_f, iota_i)

            plens_sb = constp.tile([1, BL], I32)
            hlens_sb = constp.tile([1, BL], I32)
            nc.sync.dma_start(out=plens_sb, in_=plens)
            nc.sync.dma_start(out=hlens_sb, in_=hlens)
            plens_f = constp.tile([1, BL], FP32)
            hlens_f = constp.tile([1, BL], FP32)
            nc.vector.tensor_copy(plens_f, plens_sb)
            nc.vector.tensor_copy(hlens_f, hlens_sb)

            def transpose_128(src_ap, dst_ap, dtype=FP32):
                """dst[128,128] = src[128,128]^T via PE + ACT evac."""
                pst = pool_pt.tile([128, 128], FP32, tag="pt")
                nc.tensor.matmul(pst, src_ap, ident, is_transpose=True)
                nc.scalar.copy(dst_ap, pst)

            for it in range(n_iters):
                for b in range(BL):
                    sfx = f"_{it}_{b}"
                    # ---- phase 1: load naturals ----
                    Pn = []
                    Hn = []
                    for i in range(NT):
                        t = pool_a.tile([128, 1024], FP32, tag="A", name=f"pn{i}{sfx}")
                        nc.sync.dma_start(out=t, in_=prem[b, ts(i, 128), :])
                        Pn.append(t)
                    for j in range(NT):
                        t = pool_a.tile([128, 1024], FP32, tag="A", name=f"hn{j}{sfx}")
                        nc.sync.dma_start(out=t, in_=hyp[b, ts(j, 128), :])
                        Hn.append(t)

                    # ---- phase 2: build PT/HT (d on partitions) ----
                    PT = [pool_b.tile([128, 1024], MMDT, tag="B", name=f"ptd{t}{sfx}")
                          for t in range(NT)]
                    HT = [pool_b.tile([128, 1024], MMDT, tag="B", name=f"htd{t}{sfx}")
                          for t in range(NT)]
                    for i in range(NT):
                        for t in range(NT):
                            transpose_128(Pn[i][:, ts(t, 128)], PT[t][:, ts(i, 128)])
                    for j in range(NT):
                        for t in range(NT):
                            transpose_128(Hn[j][:, ts(t, 128)], HT[t][:, ts(j, 128)])

                    # ---- masks for this batch ----
                    hlen_bc = pool_sm.tile([128, 1], FP32, tag="sm", name=f"hlb{sfx}")
                    plen_bc = pool_sm.tile([128, 1], FP32, tag="sm", name=f"plb{sfx}")
                    nc.gpsimd.partition_broadcast(hlen_bc, hlens_f[0:1, b:b + 1], channels=128)
                    nc.gpsimd.partition_broadcast(plen_bc, plens_f[0:1, b:b + 1], channels=128)
                    mask_h = pool_mask.tile([128, 1024], FP32, tag="mask", name=f"mh{sfx}")
                    mask_p = pool_mask.tile([128, 1024], FP32, tag="mask", name=f"mp{sfx}")
                    # mask = (iota >= len) * NEG_BIG
                    nc.vector.tensor_scalar(mask_h, iota_f, hlen_bc, NEG_BIG,
                                            op0=ALU.is_ge, op1=ALU.mult)
                    nc.vector.tensor_scalar(mask_p, iota_f, plen_bc, NEG_BIG,
                                            op0=ALU.is_ge, op1=ALU.mult)

                    # per-batch stats
                    negm_s = pool_sm.tile([128, NT], FP32, tag="sm", name=f"nms{sfx}")
                    s_r = pool_sm.tile([128, NT], FP32, tag="sm", name=f"sr{sfx}")
                    negm_t = pool_sm.tile([128, NT], FP32, tag="sm", name=f"nmt{sfx}")
                    s_c = pool_sm.tile([128, NT], FP32, tag="sm", name=f"sc{sfx}")

                    E1 = [pool_a.tile([128, 1024], MMDT, tag="A", name=f"e1{t}{sfx}")
                          for t in range(NT)]
                    E2 = [pool_a.tile([128, 1024], MMDT, tag="A", name=f"e2{t}{sfx}")
                          for t in range(NT)]

                    def softmax_path(idx, lhsT_set, rhs_set, mask, negm, ssum, E_out):
                        """One 128-row tile of S (or T): matmul, mask, max, exp,
                        transpose into E_out column idx."""
                        ps = pool_ps.tile([128, 1024], FP32, tag="ps",
                                          name=f"ps{idx}{sfx}")
                        for blk in range(2):
                            for t in range(NT):
                                _mm(nc, ps[:, ts(blk, 512)],
                                    lhsT_set[t][:, ts(idx, 128)],
                                    rhs_set[t][:, ts(blk, 512)],
                                    start=(t == 0), stop=(t == NT - 1),
                                    fp32r=scores_fp32r)
                        ms = pool_pre.tile([128, 1024], FP32, tag="pre",
                                           name=f"ms{idx}{sfx}")
                        nc.vector.tensor_tensor(ms, ps, mask, op=ALU.add)
                        nc.vector.reduce_max(negm[:, idx:idx + 1], ms, axis=AX,
                                             negate=True)
                        ep = pool_pre.tile([128, 1024], FP32, tag="pre",
                                           name=f"ep{idx}{sfx}")
                        nc.scalar.activation(ep, ms, ACTF.Exp,
                                             bias=negm[:, idx:idx + 1],
                                             accum_out=ssum[:, idx:idx + 1])
                        for t in range(NT):
                            transpose_128(ep[:, ts(t, 128)], E_out[t][:, ts(idx, 128)])

                    # ---- phase 3a: S path (rows p, softmax over h) ----
                    for i in range(NT):
                        softmax_path(i, PT, HT, mask_h, negm_s, s_r, E1)
                    # ---- phase 3b: T path (rows h, softmax over p) ----
                    for j in range(NT):
                        softmax_path(j, HT, PT, mask_p, negm_t, s_c, E2)

                    # ---- phase 4: reload naturals ----
                    Pr = []
                    Hr = []
                    for i in range(NT):
                        t = pool_b.tile([128, 1024], MMDT, tag="B", name=f"pr{i}{sfx}")
                        nc.sync.dma_start(out=t, in_=prem[b, ts(i, 128), :].bitcast(MMDT))
                        Pr.append(t)
                    for j in range(NT):
                        t = pool_b.tile([128, 1024], MMDT, tag="B", name=f"hr{j}{sfx}")
                        nc.sync.dma_start(out=t, in_=hyp[b, ts(j, 128), :].bitcast(MMDT))
                        Hr.append(t)

                    recip_r = pool_sm.tile([128, NT], FP32, tag="sm", name=f"rr{sfx}")
                    nrecip_r = pool_sm.tile([128, NT], FP32, tag="sm", name=f"nrr{sfx}")
                    recip_c = pool_sm.tile([128, NT], FP32, tag="sm", name=f"rc{sfx}")
                    nrecip_c = pool_sm.tile([128, NT], FP32, tag="sm", name=f"nrc{sfx}")
                    nc.vector.reciprocal(recip_r, s_r)
                    nc.vector.tensor_scalar_mul(nrecip_r, recip_r, -1.0)
                    nc.vector.reciprocal(recip_c, s_c)
                    nc.vector.tensor_scalar_mul(nrecip_c, recip_c, -1.0)

                    def ctx_out(idx, E, rhs_set, nat, recip, nrecip, out_dram):
                        """ctx matmul for one output tile + fused normalize +
                        elementwise [nat - ctx, ctx * nat] + store."""
                        ps = pool_ps.tile([128, 1024], FP32, tag="ps",
                                          name=f"cps{idx}{sfx}")
                        for blk in range(2):
                            for t in range(NT):
                                _mm(nc, ps[:, ts(blk, 512)],
                                    E[t][:, ts(idx, 128)],
                                    rhs_set[t][:, ts(blk, 512)],
                                    start=(t == 0), stop=(t == NT - 1),
                                    fp32r=ctx_fp32r)
                        st = pool_st.tile([128, 2048], FP32, tag="st",
                                          name=f"st{idx}{sfx}")
                        # nat - ctx = (ps * -recip) + nat
                        natf = nat.bitcast(FP32)
                        nc.vector.scalar_tensor_tensor(
                            st[:, 0:1024], ps, nrecip[:, idx:idx + 1], natf,
                            op0=ALU.mult, op1=ALU.add)
                        # ctx * nat = (ps * recip) * nat
                        nc.vector.scalar_tensor_tensor(
                            st[:, 1024:2048], ps, recip[:, idx:idx + 1], natf,
                            op0=ALU.mult, op1=ALU.mult)
                        nc.sync.dma_start(out=out_dram, in_=st)

                    for i in range(NT):
                        ctx_out(i, E1, Hr, Pr[i], recip_r, nrecip_r,
                                r1[ts(i, 128), b, :])
                    for j in range(NT):
                        ctx_out(j, E2, Pr, Hr[j], recip_c, nrecip_c,
                                r2[b, ts(j, 128), :])

    nc.compile()
    return nc


_NC_CACHE = {}


def _get_nc():
    if "nc" not in _NC_CACHE:
        _NC_CACHE["nc"] = build_nc()
    return _NC_CACHE["nc"]


def make_in_maps(premise, premise_lens, hypothesis, hypothesis_lens):
    premise = np.asarray(premise, dtype=np.float32)
    hypothesis = np.asarray(hypothesis, dtype=np.float32)
    premise_lens = np.asarray(premise_lens, dtype=np.int32)
    hypothesis_lens = np.asarray(hypothesis_lens, dtype=np.int32)
    in_maps = []
    for c in range(N_CORES):
        sl = slice(BL * c, BL * (c + 1))
        in_maps.append({
            "prem": np.ascontiguousarray(premise[sl]),
            "hyp": np.ascontiguousarray(hypothesis[sl]),
            "plens": premise_lens[sl].reshape(1, BL),
            "hlens": hypothesis_lens[sl].reshape(1, BL),
        })
    return in_maps


def kernel(premise, premise_lens, hypothesis, hypothesis_lens):
    nc = _get_nc()
    in_maps = make_in_maps(premise, premise_lens, hypothesis, hypothesis_lens)
    res = bass_utils.run_bass_kernel_spmd(nc, in_maps, core_ids=list(range(N_CORES)))
    r1 = np.concatenate([res.results[c]["r1"] for c in range(N_CORES)], axis=1)
    r2 = np.concatenate([res.results[c]["r2"] for c in range(N_CORES)], axis=0)
    return r1, r2
